# revision 15
# baseline (speedup 1.0000x reference)
"""Bidirectional GRU Bass kernel builder for TRN2.

Problem: B=64, L=1024, IN=H=512, bidirectional GRU (torch GRUCell semantics),
mask = ones (per spec fill), output concat([lr, reversed(rl)], axis=2).

Sharding: data-parallel over batch. Each of 8 cores handles B_SH=8 sequences,
both directions. SPMD: identical program, different input shards.

Per-core layout ("transposed domain"):
  - hidden state lives as bf16 history hist[p, kc, s, b] = h_s[b, 128*kc + p];
    the matmul stationary for step s+1 is hist[:, kc, s, :] read in place.
  - recurrent matmul: col-tiled 32-strips (strip a <- tile_position (0,32a)),
    one PSUM bank per (direction, gate r/z/n) so the six accumulation chains
    run concurrently (a bank admits one pending accumulation group at a time).
  - gate elementwise ops operate on [128, chunks, 8] tiles (transposed domain).
  - input projection per T-step chunk: feats ships as [L, B, IN] bf16 so the
    x^T staging is pure DMA-xbar-transposes; proj matmul is bf16.
  - output: history bf16 -> DMA-xbar-transpose -> SWDGE cast-DMA to fp32 DRAM.
"""

from contextlib import ExitStack

import numpy as np

import concourse.bass as bass
import concourse.mybir as mybir
import concourse.tile as tile
from concourse._compat import with_exitstack

F32 = mybir.dt.float32
BF16 = mybir.dt.bfloat16

IN = 512
H = 512
G = 3 * H  # 1536
KC = 4     # k chunks of 128 (contraction over H or IN)
MC = 12    # gate chunks of 128 (3H)
B_SH = 8   # batch per core


def prep_inputs(feats, w_ih_lr, w_hh_lr, b_ih_lr, b_hh_lr,
                w_ih_rl, w_hh_rl, b_ih_rl, b_hh_rl, n_cores=8):
    """Host-side: shard feats, arrange weights for the kernel layout.

    Returns list of per-core input dicts."""
    import ml_dtypes
    feats = np.asarray(feats, dtype=np.float32)
    B = feats.shape[0]
    bs = B // n_cores

    def arrange_w(w):  # [G, K] -> [KC, 128, G] : wT[kc, p, g] = w[g, 128*kc+p]
        w = np.asarray(w, dtype=np.float32)
        return np.ascontiguousarray(w.T.reshape(KC, 128, G))

    def arrange_whh_coltile(w):
        # [G, K] -> [KC, 128, G] with gate columns permuted for the 4x
        # col-tiled scan matmul + 32x32 stream transpose:
        # arranged col (a*384 + 32*j + u) holds std col
        #   512*(j//4) + 128*(j%4) + 32*a + u
        w = np.asarray(w, dtype=np.float32)
        acol = np.arange(G)
        a, f = acol // 384, acol % 384
        j, u = f // 32, f % 32
        std = 512 * (j // 4) + 128 * (j % 4) + 32 * a + u
        return np.ascontiguousarray(w.T.reshape(KC, 128, G)[:, :, std])

    def arrange_gxbias(b_ih, b_hh):  # [128, MC]
        b = np.asarray(b_ih, dtype=np.float32).copy()
        b[:2 * H] += np.asarray(b_hh, dtype=np.float32)[:2 * H]
        return np.ascontiguousarray(b.reshape(MC, 128).T)

    def arrange_bhn(b_hh):  # [128, 4]
        return np.ascontiguousarray(
            np.asarray(b_hh, dtype=np.float32)[2 * H:].reshape(4, 128).T)

    shared = {
        'whhT': np.stack([arrange_whh_coltile(w_hh_lr),
                          arrange_whh_coltile(w_hh_rl)]).astype(ml_dtypes.bfloat16),
        'wihT': np.stack([arrange_w(w_ih_lr),
                          arrange_w(w_ih_rl)]).astype(ml_dtypes.bfloat16),
        'gxbias': np.stack([arrange_gxbias(b_ih_lr, b_hh_lr),
                            arrange_gxbias(b_ih_rl, b_hh_rl)]),
        'bhn': np.stack([arrange_bhn(b_hh_lr), arrange_bhn(b_hh_rl)]),
    }
    in_maps = []
    for c in range(n_cores):
        m = dict(shared)
        # [L, B_SH, IN] bf16: (token, batch) rows contiguous for xbar transpose
        m['feats'] = np.ascontiguousarray(
            feats[c * bs:(c + 1) * bs].transpose(1, 0, 2)).astype(
                ml_dtypes.bfloat16)
        in_maps.append(m)
    return in_maps


@with_exitstack
def gru_core_kernel(ctx: ExitStack, tc: tile.TileContext,
                    out_ap: bass.AP, feats: bass.AP, whhT: bass.AP,
                    wihT: bass.AP, gxbias: bass.AP, bhn: bass.AP,
                    L: int, T: int):
    nc = tc.nc
    NCH = L // T
    NTG = T // 16            # 16-token groups per chunk (DMA/transpose granularity)
    TOK = T * B_SH           # tokens per chunk per direction

    singles = ctx.enter_context(tc.tile_pool(name="singles", bufs=1))
    xtpool = ctx.enter_context(tc.tile_pool(name="xtpool", bufs=1))
    gxpool = ctx.enter_context(tc.tile_pool(name="gxpool", bufs=2))
    histpool = ctx.enter_context(tc.tile_pool(name="histpool", bufs=2))
    outpool = ctx.enter_context(tc.tile_pool(name="outpool", bufs=2))
    scratch = ctx.enter_context(tc.tile_pool(name="scratch", bufs=3))
    scan_ps = ctx.enter_context(tc.tile_pool(name="scan_ps", bufs=1, space="PSUM"))
    proj_ps = ctx.enter_context(tc.tile_pool(name="proj_ps", bufs=2, space="PSUM"))

    # --- persistent weights / biases in SBUF ---
    whh_sb, wih_sb, gxb_sb, bhn_sb, h0 = [], [], [], [], []
    for d in range(2):
        wh = singles.tile([128, KC, G], BF16, tag=f"whh{d}", name=f"whh{d}")
        nc.sync.dma_start(wh[:], whhT[d].rearrange("kc p g -> p kc g"))
        whh_sb.append(wh)
        wi = singles.tile([128, KC, G], BF16, tag=f"wih{d}", name=f"wih{d}")
        nc.sync.dma_start(wi[:], wihT[d].rearrange("kc p g -> p kc g"))
        wih_sb.append(wi)
        gb = singles.tile([128, MC], F32, tag=f"gxb{d}", name=f"gxb{d}")
        nc.sync.dma_start(gb[:], gxbias[d])
        gxb_sb.append(gb)
        bh = singles.tile([128, 4], F32, tag=f"bhn{d}", name=f"bhn{d}")
        nc.sync.dma_start(bh[:], bhn[d])
        bhn_sb.append(bh)
        # zero initial hidden state, indexed like a 1-slot history tile.
        hz = singles.tile([128, KC, 1, B_SH], BF16, tag=f"h0{d}", name=f"h0{d}")
        nc.vector.memset(hz[:], 0.0)
        h0.append(hz)

    # Scan psum tiles, two parity banks per direction. Pre-zeroed: the
    # 8-column stationary writes only partitions 32a..32a+8 of each 32-strip;
    # lanes 8..32 must read as zeros in the stream-transposes forever after.
    pst_tiles = []
    for d in range(2):
        pair = []
        for i in range(2):
            p = scan_ps.tile([128, 384], F32, tag=f"pst{d}{i}",
                             name=f"pst{d}{i}")
            nc.vector.memset(p[:], 0.0)
            pair.append(p)
        pst_tiles.append(pair)

    def x_rows(d, c):
        # chunk c, dir d: DRAM time-window rows [w0, w0+T)
        return c * T if d == 0 else L - (c + 1) * T

    def make_proj(d, c):
        """Build gxT tile + list of emission thunks (x^T DMA-transposes, proj).

        Thunks are emitted interleaved with the previous chunk's scan steps so
        proj matmuls fill the PE's dependency-wait gaps."""
        w0 = x_rows(d, c)
        xT = xtpool.tile([128, KC, TOK], BF16, tag=f"xT{d}", name=f"xT{d}")
        gxT = gxpool.tile([128, MC, T, B_SH], F32, tag=f"gx{d}", name=f"gx{d}")
        thunks = []

        def tr_thunk(g, kc):
            def f():
                # xbar transpose straight from DRAM: rows = 128 contiguous
                # (token, batch) pairs, cols = one 128-wide feature chunk.
                nc.sync.dma_start_transpose(
                    out=xT[:, kc, 128 * g:128 * (g + 1)],
                    in_=feats[w0 + 16 * g:w0 + 16 * (g + 1), :,
                              128 * kc:128 * (kc + 1)]
                    .rearrange("s b d -> (s b) d"))
            return f

        def mm_thunk(mc):
            def f():
                pj = proj_ps.tile([128, TOK], F32, tag="pj", name="pj")
                for kc in range(KC):
                    nc.tensor.matmul(
                        pj[:],
                        lhsT=wih_sb[d][:, kc, 128 * mc:128 * (mc + 1)],
                        rhs=xT[:, kc, :],
                        start=(kc == 0), stop=(kc == KC - 1))
                nc.scalar.add(out=gxT[:, mc, :, :],
                              in_=pj[:].rearrange("p (t b) -> p t b", b=B_SH),
                              add=gxb_sb[d][:, mc:mc + 1])
            return f

        for g in range(NTG):
            for kc in range(KC):
                thunks.append(tr_thunk(g, kc))
        for mc in range(MC):
            thunks.append(mm_thunk(mc))
        return gxT, thunks

    def scan_mms(t, h_prev_of, s_prev_of):
        """Emit both directions' recurrent matmuls for one step.

        Strip-serial (the PE executes matmuls at a serial stream cadence;
        col-strips share all row groups so their LDWEIGHTS cannot overlap
        in-flight matmuls). Stationary = previous step's bf16 h history slot,
        read in place.
        """
        for d in range(2):
            pst = pst_tiles[d][t % 2]
            for a in range(4):
                for kc in range(KC):
                    nc.tensor.matmul(
                        pst[32 * a:32 * a + B_SH, :],
                        lhsT=h_prev_of[d][:, kc, s_prev_of[d], :],
                        rhs=whh_sb[d][:, kc, 384 * a:384 * (a + 1)],
                        start=(kc == 0), stop=(kc == KC - 1),
                        tile_position=(0, 32 * a))

    def scan_gates(d, gxT, histT, s, t, h_prev_tile, s_prev):
        pst = pst_tiles[d][t % 2]
        # 32x32 block stream-transpose -> ghT[p, j, lane]; lanes 0:8 real.
        # (p, j) maps to std gate col 512*(j//4) + 128*(j%4) + p.
        # Split rz / n so sigmoid can start before the n-part transpose.
        ghT = scratch.tile([128, MC, 32], F32, tag=f"ghT{d}", name=f"ghT{d}")
        nc.vector.transpose(out=ghT[:, 0:8, :].rearrange("p j u -> p (j u)"),
                            in_=pst[:, 0:256])
        srz = scratch.tile([128, 8, B_SH], F32, tag=f"srz{d}", name=f"srz{d}")
        nc.vector.tensor_tensor(out=srz[:], in0=ghT[:, 0:8, 0:B_SH],
                                in1=gxT[:, 0:8, s, :], op=mybir.AluOpType.add)
        rz = scratch.tile([128, 8, B_SH], F32, tag=f"rz{d}", name=f"rz{d}")
        nc.scalar.activation(out=rz[:], in_=srz[:],
                             func=mybir.ActivationFunctionType.Sigmoid)
        nc.vector.transpose(out=ghT[:, 8:12, :].rearrange("p j u -> p (j u)"),
                            in_=pst[:, 256:384])
        # v = ghn + bhn ; w = v*r ; u = w + gxn ; n = tanh(u)
        # v is off the critical path (slack while sigmoid runs) -> GpSimd.
        v = scratch.tile([128, 4, B_SH], F32, tag=f"v{d}", name=f"v{d}")
        nc.gpsimd.tensor_tensor(out=v[:], in0=ghT[:, 8:12, 0:B_SH],
                                in1=bhn_sb[d][:, :, None].to_broadcast((128, 4, B_SH)),
                                op=mybir.AluOpType.add)
        nc.vector.tensor_tensor(out=v[:], in0=v[:], in1=rz[:, 0:4, :],
                                op=mybir.AluOpType.mult)
        nc.vector.tensor_tensor(out=v[:], in0=v[:], in1=gxT[:, 8:12, s, :],
                                op=mybir.AluOpType.add)
        n = scratch.tile([128, 4, B_SH], F32, tag=f"n{d}", name=f"n{d}")
        nc.scalar.activation(out=n[:], in_=v[:],
                             func=mybir.ActivationFunctionType.Tanh)
        # h' = n + z*(h - n)   (mask==1 so h=h' always); h' lands straight in
        # its bf16 history slot, which doubles as the next step's matmul
        # stationary — no staging copies at all.
        dlt = scratch.tile([128, 4, B_SH], F32, tag=f"dlt{d}", name=f"dlt{d}")
        nc.vector.tensor_tensor(out=dlt[:], in0=h_prev_tile[:, :, s_prev, :],
                                in1=n[:], op=mybir.AluOpType.subtract)
        nc.vector.tensor_tensor(out=dlt[:], in0=dlt[:], in1=rz[:, 4:8, :],
                                op=mybir.AluOpType.mult)
        nc.vector.tensor_tensor(out=histT[:, :, s, :], in0=n[:], in1=dlt[:],
                                op=mybir.AluOpType.add)

    def out_chunk(d, c, histT):
        w0 = x_rows(d, c)
        # hist (bf16, h-major) -> xbar transpose -> ostb rows=(s,b) tokens,
        # then one SWDGE cast-DMA (bf16 -> fp32) per 16-token group to DRAM.
        ostb = outpool.tile([128, NTG, KC, 128], BF16, tag=f"ost{d}",
                            name=f"ost{d}")
        for g in range(NTG):
            for kc in range(KC):
                nc.sync.dma_start_transpose(
                    out=ostb[:, g, kc, :],
                    in_=histT[:, kc, 16 * g:16 * (g + 1), :]
                    .rearrange("p s b -> p (s b)"))
        c0 = 0 if d == 0 else H
        for g in range(NTG):
            nc.gpsimd.dma_start(
                out=out_ap[:, w0 + 16 * g:w0 + 16 * (g + 1), c0:c0 + H]
                .rearrange("b s h -> s b h"),
                in_=ostb[:, g, :, :])

    # --- software-pipelined chunk loop: next chunk's proj thunks are emitted
    # interleaved with this chunk's scan steps ---
    first = [make_proj(d, 0) for d in range(2)]
    gx_cur = [p[0] for p in first]
    for _, ths in first:
        for th in ths:
            th()
    for c in range(NCH):
        if c + 1 < NCH:
            nxt = [make_proj(d, c + 1) for d in range(2)]
            pending = [th for pair in zip(nxt[0][1], nxt[1][1]) for th in pair]
            gx_next = [nxt[0][0], nxt[1][0]]
        else:
            pending, gx_next = [], None
        hist = [histpool.tile([128, KC, T, B_SH], BF16, tag=f"hist{d}", name=f"hist{d}")
                for d in range(2)]
        per = max(1, -(-len(pending) // T)) if pending else 0
        for t in range(T):
            h_prev_of, s_prev_of, s_of = [], [], []
            for d in range(2):
                # rl consumes its (forward-loaded) chunk in reverse slot order
                s = t if d == 0 else T - 1 - t
                if t == 0:
                    if c == 0:
                        h_prev_tile, s_prev = h0[d], 0
                    else:
                        h_prev_tile = hist_prev[d]
                        s_prev = T - 1 if d == 0 else 0
                else:
                    h_prev_tile = hist[d]
                    s_prev = s - 1 if d == 0 else s + 1
                h_prev_of.append(h_prev_tile)
                s_prev_of.append(s_prev)
                s_of.append(s)
            scan_mms(t, h_prev_of, s_prev_of)
            for d in range(2):
                scan_gates(d, gx_cur[d], hist[d], s_of[d], t, h_prev_of[d],
                           s_prev_of[d])
            for _ in range(per):
                if pending:
                    pending.pop(0)()
        while pending:
            pending.pop(0)()
        for d in range(2):
            out_chunk(d, c, hist[d])
        gx_cur = gx_next
        hist_prev = hist


def build_nc(L=1024, T=32, num_devices=8, debug=False):
    from concourse import bacc
    nc = bacc.Bacc("TRN2", target_bir_lowering=False, debug=debug,
                   enable_asserts=True, num_devices=num_devices)
    feats = nc.dram_tensor("feats", [L, B_SH, IN], BF16, kind="ExternalInput").ap()
    whhT = nc.dram_tensor("whhT", [2, KC, 128, G], BF16, kind="ExternalInput").ap()
    wihT = nc.dram_tensor("wihT", [2, KC, 128, G], BF16, kind="ExternalInput").ap()
    gxbias = nc.dram_tensor("gxbias", [2, 128, MC], F32, kind="ExternalInput").ap()
    bhn = nc.dram_tensor("bhn", [2, 128, 4], F32, kind="ExternalInput").ap()
    out = nc.dram_tensor("out", [B_SH, L, 2 * H], F32, kind="ExternalOutput").ap()
    with tile.TileContext(nc) as tc:
        gru_core_kernel(tc, out, feats, whhT, wihT, gxbias, bhn, L, T)
    nc.compile()
    return nc


# ---------------------------------------------------------------------------
# Self-contained harness entry point: kernel(**inputs) -> np.ndarray
# ---------------------------------------------------------------------------

N_CORES = 8
L_FULL = 1024
T_CHUNK = 32

_STATE = {}


def _get_exec():
    if 'fn' in _STATE:
        return _STATE
    import jax
    from jax.sharding import Mesh, PartitionSpec, NamedSharding
    from jax.experimental.shard_map import shard_map
    from concourse.bass2jax import (_bass_exec_p, install_neuronx_cc_hook,
                                    partition_id_tensor)

    nc = build_nc(L=L_FULL, T=T_CHUNK, num_devices=N_CORES)
    install_neuronx_cc_hook()
    partition_name = nc.partition_id_tensor.name if nc.partition_id_tensor else None

    in_names, out_names, out_avals = [], [], []
    for alloc in nc.m.functions[0].allocations:
        if not isinstance(alloc, mybir.MemoryLocationSet):
            continue
        name = alloc.memorylocations[0].name
        if alloc.kind == "ExternalInput":
            if name != partition_name:
                in_names.append(name)
        elif alloc.kind == "ExternalOutput":
            out_names.append(name)
            out_avals.append(jax.core.ShapedArray(
                tuple(alloc.tensor_shape), mybir.dt.np(alloc.dtype)))
    all_in_names = list(in_names) + list(out_names)
    if partition_name is not None:
        all_in_names.append(partition_name)

    def _body(*args):
        operands = list(args)
        if partition_name is not None:
            operands.append(partition_id_tensor())
        return tuple(_bass_exec_p.bind(
            *operands, out_avals=tuple(out_avals), in_names=tuple(all_in_names),
            out_names=tuple(out_names), lowering_input_output_aliases=(),
            sim_require_finite=True, sim_require_nnan=True, nc=nc))

    devices = jax.devices()[:N_CORES]
    mesh = Mesh(np.asarray(devices), ("core",))
    spec = PartitionSpec("core")
    n_in = len(in_names) + len(out_avals)
    fn = jax.jit(shard_map(_body, mesh=mesh, in_specs=(spec,) * n_in,
                           out_specs=(spec,) * len(out_names), check_rep=False),
                 keep_unused=True)
    _STATE.update(fn=fn, in_names=in_names, out_names=out_names,
                  out_avals=out_avals, mesh=mesh, spec=spec)
    return _STATE


def _stage_inputs(in_maps):
    import jax
    from jax.sharding import NamedSharding
    st = _get_exec()
    sh = NamedSharding(st['mesh'], st['spec'])
    args = []
    for nm in st['in_names']:
        a = np.concatenate([np.asarray(in_maps[c][nm]) for c in range(N_CORES)],
                           axis=0)
        args.append(jax.device_put(a, sh))
    for av in st['out_avals']:
        z = np.zeros((N_CORES * av.shape[0], *av.shape[1:]), av.dtype)
        args.append(jax.device_put(z, sh))
    return args


def _run(args):
    import jax
    st = _get_exec()
    outs = st['fn'](*args)
    jax.block_until_ready(outs)
    return outs


def kernel(feats, feats_mask, w_ih_lr, w_hh_lr, b_ih_lr, b_hh_lr,
           w_ih_rl, w_hh_rl, b_ih_rl, b_hh_rl):
    """Full-input bidirectional GRU on 8 NeuronCores (batch data-parallel).

    feats_mask is all-ones for this problem spec and is not used on device.
    """
    in_maps = prep_inputs(feats, w_ih_lr, w_hh_lr, b_ih_lr, b_hh_lr,
                          w_ih_rl, w_hh_rl, b_ih_rl, b_hh_rl, n_cores=N_CORES)
    args = _stage_inputs(in_maps)
    outs = _run(args)
    st = _STATE
    oi = st['out_names'].index('out')
    full = np.asarray(outs[oi])  # [N_CORES*B_SH, L, 2H] (batch-concat)
    return full


# revision 16
# speedup vs baseline: 1.3970x; 1.3970x over previous
"""Bidirectional GRU Bass kernel builder for TRN2.

Problem: B=64, L=1024, IN=H=512, bidirectional GRU (torch GRUCell semantics),
mask = ones (per spec fill), output concat([lr, reversed(rl)], axis=2).

Sharding: data-parallel over batch. Each of 8 cores handles B_SH=8 sequences,
both directions. SPMD: identical program, different input shards.

Per-core layout ("transposed domain"):
  - hidden state lives as bf16 history hist[p, kc, s, b] = h_s[b, 128*kc + p];
    the matmul stationary for step s+1 is hist[:, kc, s, :] read in place.
  - recurrent matmul: col-tiled 32-strips (strip a <- tile_position (0,32a)),
    one PSUM bank per (direction, gate r/z/n) so the six accumulation chains
    run concurrently (a bank admits one pending accumulation group at a time).
  - gate elementwise ops operate on [128, chunks, 8] tiles (transposed domain).
  - input projection per T-step chunk: feats ships as [L, B, IN] bf16 so the
    x^T staging is pure DMA-xbar-transposes; proj matmul is bf16.
  - output: history bf16 -> DMA-xbar-transpose -> SWDGE cast-DMA to fp32 DRAM.
"""

from contextlib import ExitStack

import numpy as np

import concourse.bass as bass
import concourse.mybir as mybir
import concourse.tile as tile
from concourse._compat import with_exitstack

F32 = mybir.dt.float32
BF16 = mybir.dt.bfloat16

IN = 512
H = 512
G = 3 * H  # 1536
KC = 4     # k chunks of 128 (contraction over H or IN)
MC = 12    # gate chunks of 128 (3H)
B_SH = 8   # batch per core


def prep_inputs(feats, w_ih_lr, w_hh_lr, b_ih_lr, b_hh_lr,
                w_ih_rl, w_hh_rl, b_ih_rl, b_hh_rl, n_cores=8):
    """Host-side: shard feats, arrange weights for the kernel layout.

    Returns list of per-core input dicts."""
    import ml_dtypes
    feats = np.asarray(feats, dtype=np.float32)
    B = feats.shape[0]
    bs = B // n_cores

    def arrange_w(w):  # [G, K] -> [KC, 128, G] : wT[kc, p, g] = w[g, 128*kc+p]
        w = np.asarray(w, dtype=np.float32)
        return np.ascontiguousarray(w.T.reshape(KC, 128, G))

    def arrange_whh_coltile(w):
        # [G, K] -> [KC, 128, G] with gate columns permuted for the 4x
        # col-tiled scan matmul + 32x32 stream transpose:
        # arranged col (a*384 + 32*j + u) holds std col
        #   512*(j//4) + 128*(j%4) + 32*a + u
        w = np.asarray(w, dtype=np.float32)
        acol = np.arange(G)
        a, f = acol // 384, acol % 384
        j, u = f // 32, f % 32
        std = 512 * (j // 4) + 128 * (j % 4) + 32 * a + u
        return np.ascontiguousarray(w.T.reshape(KC, 128, G)[:, :, std])

    def arrange_gxbias(b_ih, b_hh):  # [128, MC]
        b = np.asarray(b_ih, dtype=np.float32).copy()
        b[:2 * H] += np.asarray(b_hh, dtype=np.float32)[:2 * H]
        return np.ascontiguousarray(b.reshape(MC, 128).T)

    def arrange_bhn(b_hh):  # [128, 4]
        return np.ascontiguousarray(
            np.asarray(b_hh, dtype=np.float32)[2 * H:].reshape(4, 128).T)

    shared = {
        'whhT': np.stack([arrange_whh_coltile(w_hh_lr),
                          arrange_whh_coltile(w_hh_rl)]).astype(ml_dtypes.bfloat16),
        'wihT': np.stack([arrange_w(w_ih_lr),
                          arrange_w(w_ih_rl)]).astype(ml_dtypes.bfloat16),
        'gxbias': np.stack([arrange_gxbias(b_ih_lr, b_hh_lr),
                            arrange_gxbias(b_ih_rl, b_hh_rl)]),
        'bhn': np.stack([arrange_bhn(b_hh_lr), arrange_bhn(b_hh_rl)]),
    }
    in_maps = []
    for c in range(n_cores):
        m = dict(shared)
        # [L, B_SH, IN] bf16: (token, batch) rows contiguous for xbar transpose
        m['feats'] = np.ascontiguousarray(
            feats[c * bs:(c + 1) * bs].transpose(1, 0, 2)).astype(
                ml_dtypes.bfloat16)
        in_maps.append(m)
    return in_maps


@with_exitstack
def gru_core_kernel(ctx: ExitStack, tc: tile.TileContext,
                    out_ap: bass.AP, feats: bass.AP, whhT: bass.AP,
                    wihT: bass.AP, gxbias: bass.AP, bhn: bass.AP,
                    L: int, T: int):
    nc = tc.nc
    NCH = L // T
    NTG = T // 16            # 16-token groups per chunk (DMA/transpose granularity)
    TOK = T * B_SH           # tokens per chunk per direction

    singles = ctx.enter_context(tc.tile_pool(name="singles", bufs=1))
    xtpool = ctx.enter_context(tc.tile_pool(name="xtpool", bufs=1))
    gxpool = ctx.enter_context(tc.tile_pool(name="gxpool", bufs=2))
    histpool = ctx.enter_context(tc.tile_pool(name="histpool", bufs=2))
    outpool = ctx.enter_context(tc.tile_pool(name="outpool", bufs=2))
    scratch = ctx.enter_context(tc.tile_pool(name="scratch", bufs=3))
    scan_ps = ctx.enter_context(tc.tile_pool(name="scan_ps", bufs=1, space="PSUM"))
    proj_ps = ctx.enter_context(tc.tile_pool(name="proj_ps", bufs=2, space="PSUM"))

    # --- persistent weights / biases in SBUF ---
    whh_sb, wih_sb, gxb_sb, bhn_sb, h0 = [], [], [], [], []
    for d in range(2):
        wh = singles.tile([128, KC, G], BF16, tag=f"whh{d}", name=f"whh{d}")
        nc.sync.dma_start(wh[:], whhT[d].rearrange("kc p g -> p kc g"))
        whh_sb.append(wh)
        wi = singles.tile([128, KC, G], BF16, tag=f"wih{d}", name=f"wih{d}")
        nc.sync.dma_start(wi[:], wihT[d].rearrange("kc p g -> p kc g"))
        wih_sb.append(wi)
        gb = singles.tile([128, MC], F32, tag=f"gxb{d}", name=f"gxb{d}")
        nc.sync.dma_start(gb[:], gxbias[d])
        gxb_sb.append(gb)
        bh = singles.tile([128, 4], F32, tag=f"bhn{d}", name=f"bhn{d}")
        nc.sync.dma_start(bh[:], bhn[d])
        bhn_sb.append(bh)
        # zero initial hidden state, indexed like a 1-slot history tile.
        hz = singles.tile([128, KC, 1, B_SH], BF16, tag=f"h0{d}", name=f"h0{d}")
        nc.vector.memset(hz[:], 0.0)
        h0.append(hz)

    # Scan psum tiles, two parity banks per direction. Pre-zeroed: the
    # 8-column stationary writes only partitions 32a..32a+8 of each 32-strip;
    # lanes 8..32 must read as zeros in the stream-transposes forever after.
    pst_tiles = []
    for d in range(2):
        pair = []
        for i in range(2):
            p = scan_ps.tile([128, 384], F32, tag=f"pst{d}{i}",
                             name=f"pst{d}{i}")
            nc.vector.memset(p[:], 0.0)
            pair.append(p)
        pst_tiles.append(pair)

    def x_rows(d, c):
        # chunk c, dir d: DRAM time-window rows [w0, w0+T)
        return c * T if d == 0 else L - (c + 1) * T

    def make_proj(d, c):
        """Build gxT tile + list of emission thunks (x^T DMA-transposes, proj).

        Thunks are emitted interleaved with the previous chunk's scan steps so
        proj matmuls fill the PE's dependency-wait gaps."""
        w0 = x_rows(d, c)
        xT = xtpool.tile([128, KC, TOK], BF16, tag=f"xT{d}", name=f"xT{d}")
        gxT = gxpool.tile([128, MC, T, B_SH], F32, tag=f"gx{d}", name=f"gx{d}")
        thunks = []

        def tr_thunk(g, kc):
            def f():
                # xbar transpose straight from DRAM: rows = 128 contiguous
                # (token, batch) pairs, cols = one 128-wide feature chunk.
                nc.sync.dma_start_transpose(
                    out=xT[:, kc, 128 * g:128 * (g + 1)],
                    in_=feats[w0 + 16 * g:w0 + 16 * (g + 1), :,
                              128 * kc:128 * (kc + 1)]
                    .rearrange("s b d -> (s b) d"))
            return f

        def mm_thunk(mc):
            def f():
                pj = proj_ps.tile([128, TOK], F32, tag="pj", name="pj")
                for kc in range(KC):
                    nc.tensor.matmul(
                        pj[:],
                        lhsT=wih_sb[d][:, kc, 128 * mc:128 * (mc + 1)],
                        rhs=xT[:, kc, :],
                        start=(kc == 0), stop=(kc == KC - 1))
                nc.scalar.add(out=gxT[:, mc, :, :],
                              in_=pj[:].rearrange("p (t b) -> p t b", b=B_SH),
                              add=gxb_sb[d][:, mc:mc + 1])
            return f

        for g in range(NTG):
            for kc in range(KC):
                thunks.append(tr_thunk(g, kc))
        for mc in range(MC):
            thunks.append(mm_thunk(mc))
        return gxT, thunks

    def scan_mms(t, h_prev_of, s_prev_of):
        """Emit both directions' recurrent matmuls for one step.

        Strip-serial (the PE executes matmuls at a serial stream cadence;
        col-strips share all row groups so their LDWEIGHTS cannot overlap
        in-flight matmuls). Stationary = previous step's bf16 h history slot,
        read in place.
        """
        for d in range(2):
            pst = pst_tiles[d][t % 2]
            for a in range(4):
                for kc in range(KC):
                    nc.tensor.matmul(
                        pst[32 * a:32 * a + B_SH, :],
                        lhsT=h_prev_of[d][:, kc, s_prev_of[d], :],
                        rhs=whh_sb[d][:, kc, 384 * a:384 * (a + 1)],
                        start=(kc == 0), stop=(kc == KC - 1),
                        tile_position=(0, 32 * a))

    def scan_gates(d, gxT, histT, s, t, h_prev_tile, s_prev):
        pst = pst_tiles[d][t % 2]
        # 32x32 block stream-transpose -> ghT[p, j, lane]; lanes 0:8 real.
        # (p, j) maps to std gate col 512*(j//4) + 128*(j%4) + p.
        # Split rz / n so sigmoid can start before the n-part transpose.
        ghT = scratch.tile([128, MC, 32], F32, tag=f"ghT{d}", name=f"ghT{d}")
        nc.vector.transpose(out=ghT[:, 0:8, :].rearrange("p j u -> p (j u)"),
                            in_=pst[:, 0:256])
        srz = scratch.tile([128, 8, B_SH], F32, tag=f"srz{d}", name=f"srz{d}")
        nc.vector.tensor_tensor(out=srz[:], in0=ghT[:, 0:8, 0:B_SH],
                                in1=gxT[:, 0:8, s, :], op=mybir.AluOpType.add)
        rz = scratch.tile([128, 8, B_SH], F32, tag=f"rz{d}", name=f"rz{d}")
        nc.scalar.activation(out=rz[:], in_=srz[:],
                             func=mybir.ActivationFunctionType.Sigmoid)
        nc.vector.transpose(out=ghT[:, 8:12, :].rearrange("p j u -> p (j u)"),
                            in_=pst[:, 256:384])
        # v = ghn + bhn ; w = v*r ; u = w + gxn ; n = tanh(u)
        v = scratch.tile([128, 4, B_SH], F32, tag=f"v{d}", name=f"v{d}")
        nc.vector.tensor_tensor(out=v[:], in0=ghT[:, 8:12, 0:B_SH],
                                in1=bhn_sb[d][:, :, None].to_broadcast((128, 4, B_SH)),
                                op=mybir.AluOpType.add)
        nc.vector.tensor_tensor(out=v[:], in0=v[:], in1=rz[:, 0:4, :],
                                op=mybir.AluOpType.mult)
        nc.vector.tensor_tensor(out=v[:], in0=v[:], in1=gxT[:, 8:12, s, :],
                                op=mybir.AluOpType.add)
        n = scratch.tile([128, 4, B_SH], F32, tag=f"n{d}", name=f"n{d}")
        nc.scalar.activation(out=n[:], in_=v[:],
                             func=mybir.ActivationFunctionType.Tanh)
        # h' = n + z*(h - n)   (mask==1 so h=h' always); h' lands straight in
        # its bf16 history slot, which doubles as the next step's matmul
        # stationary — no staging copies at all.
        dlt = scratch.tile([128, 4, B_SH], F32, tag=f"dlt{d}", name=f"dlt{d}")
        nc.vector.tensor_tensor(out=dlt[:], in0=h_prev_tile[:, :, s_prev, :],
                                in1=n[:], op=mybir.AluOpType.subtract)
        nc.vector.tensor_tensor(out=dlt[:], in0=dlt[:], in1=rz[:, 4:8, :],
                                op=mybir.AluOpType.mult)
        nc.vector.tensor_tensor(out=histT[:, :, s, :], in0=n[:], in1=dlt[:],
                                op=mybir.AluOpType.add)

    def out_chunk(d, c, histT):
        w0 = x_rows(d, c)
        # hist (bf16, h-major) -> xbar transpose -> ostb rows=(s,b) tokens,
        # then one SWDGE cast-DMA (bf16 -> fp32) per 16-token group to DRAM.
        ostb = outpool.tile([128, NTG, KC, 128], BF16, tag=f"ost{d}",
                            name=f"ost{d}")
        for g in range(NTG):
            for kc in range(KC):
                nc.sync.dma_start_transpose(
                    out=ostb[:, g, kc, :],
                    in_=histT[:, kc, 16 * g:16 * (g + 1), :]
                    .rearrange("p s b -> p (s b)"))
        c0 = 0 if d == 0 else H
        for g in range(NTG):
            nc.gpsimd.dma_start(
                out=out_ap[:, w0 + 16 * g:w0 + 16 * (g + 1), c0:c0 + H]
                .rearrange("b s h -> s b h"),
                in_=ostb[:, g, :, :])

    # --- software-pipelined chunk loop: next chunk's proj thunks are emitted
    # interleaved with this chunk's scan steps ---
    first = [make_proj(d, 0) for d in range(2)]
    gx_cur = [p[0] for p in first]
    for _, ths in first:
        for th in ths:
            th()
    for c in range(NCH):
        if c + 1 < NCH:
            nxt = [make_proj(d, c + 1) for d in range(2)]
            pending = [th for pair in zip(nxt[0][1], nxt[1][1]) for th in pair]
            gx_next = [nxt[0][0], nxt[1][0]]
        else:
            pending, gx_next = [], None
        hist = [histpool.tile([128, KC, T, B_SH], BF16, tag=f"hist{d}", name=f"hist{d}")
                for d in range(2)]
        per = max(1, -(-len(pending) // T)) if pending else 0
        for t in range(T):
            h_prev_of, s_prev_of, s_of = [], [], []
            for d in range(2):
                # rl consumes its (forward-loaded) chunk in reverse slot order
                s = t if d == 0 else T - 1 - t
                if t == 0:
                    if c == 0:
                        h_prev_tile, s_prev = h0[d], 0
                    else:
                        h_prev_tile = hist_prev[d]
                        s_prev = T - 1 if d == 0 else 0
                else:
                    h_prev_tile = hist[d]
                    s_prev = s - 1 if d == 0 else s + 1
                h_prev_of.append(h_prev_tile)
                s_prev_of.append(s_prev)
                s_of.append(s)
            scan_mms(t, h_prev_of, s_prev_of)
            for d in range(2):
                scan_gates(d, gx_cur[d], hist[d], s_of[d], t, h_prev_of[d],
                           s_prev_of[d])
            for _ in range(per):
                if pending:
                    pending.pop(0)()
        while pending:
            pending.pop(0)()
        for d in range(2):
            out_chunk(d, c, hist[d])
        gx_cur = gx_next
        hist_prev = hist


def build_nc(L=1024, T=32, num_devices=8, debug=False):
    from concourse import bacc
    nc = bacc.Bacc("TRN2", target_bir_lowering=False, debug=debug,
                   enable_asserts=True, num_devices=num_devices)
    feats = nc.dram_tensor("feats", [L, B_SH, IN], BF16, kind="ExternalInput").ap()
    whhT = nc.dram_tensor("whhT", [2, KC, 128, G], BF16, kind="ExternalInput").ap()
    wihT = nc.dram_tensor("wihT", [2, KC, 128, G], BF16, kind="ExternalInput").ap()
    gxbias = nc.dram_tensor("gxbias", [2, 128, MC], F32, kind="ExternalInput").ap()
    bhn = nc.dram_tensor("bhn", [2, 128, 4], F32, kind="ExternalInput").ap()
    out = nc.dram_tensor("out", [B_SH, L, 2 * H], F32, kind="ExternalOutput").ap()
    with tile.TileContext(nc) as tc:
        gru_core_kernel(tc, out, feats, whhT, wihT, gxbias, bhn, L, T)
    nc.compile()
    return nc


# ---------------------------------------------------------------------------
# Self-contained harness entry point: kernel(**inputs) -> np.ndarray
# ---------------------------------------------------------------------------

N_CORES = 8
L_FULL = 1024
T_CHUNK = 32

_STATE = {}


def _get_exec():
    if 'fn' in _STATE:
        return _STATE
    import jax
    from jax.sharding import Mesh, PartitionSpec, NamedSharding
    from jax.experimental.shard_map import shard_map
    from concourse.bass2jax import (_bass_exec_p, install_neuronx_cc_hook,
                                    partition_id_tensor)

    nc = build_nc(L=L_FULL, T=T_CHUNK, num_devices=N_CORES)
    install_neuronx_cc_hook()
    partition_name = nc.partition_id_tensor.name if nc.partition_id_tensor else None

    in_names, out_names, out_avals = [], [], []
    for alloc in nc.m.functions[0].allocations:
        if not isinstance(alloc, mybir.MemoryLocationSet):
            continue
        name = alloc.memorylocations[0].name
        if alloc.kind == "ExternalInput":
            if name != partition_name:
                in_names.append(name)
        elif alloc.kind == "ExternalOutput":
            out_names.append(name)
            out_avals.append(jax.core.ShapedArray(
                tuple(alloc.tensor_shape), mybir.dt.np(alloc.dtype)))
    all_in_names = list(in_names) + list(out_names)
    if partition_name is not None:
        all_in_names.append(partition_name)

    def _body(*args):
        operands = list(args)
        if partition_name is not None:
            operands.append(partition_id_tensor())
        return tuple(_bass_exec_p.bind(
            *operands, out_avals=tuple(out_avals), in_names=tuple(all_in_names),
            out_names=tuple(out_names), lowering_input_output_aliases=(),
            sim_require_finite=True, sim_require_nnan=True, nc=nc))

    devices = jax.devices()[:N_CORES]
    mesh = Mesh(np.asarray(devices), ("core",))
    spec = PartitionSpec("core")
    n_in = len(in_names) + len(out_avals)
    fn = jax.jit(shard_map(_body, mesh=mesh, in_specs=(spec,) * n_in,
                           out_specs=(spec,) * len(out_names), check_rep=False),
                 keep_unused=True)
    _STATE.update(fn=fn, in_names=in_names, out_names=out_names,
                  out_avals=out_avals, mesh=mesh, spec=spec)
    return _STATE


def _stage_inputs(in_maps):
    import jax
    from jax.sharding import NamedSharding
    st = _get_exec()
    sh = NamedSharding(st['mesh'], st['spec'])
    args = []
    for nm in st['in_names']:
        a = np.concatenate([np.asarray(in_maps[c][nm]) for c in range(N_CORES)],
                           axis=0)
        args.append(jax.device_put(a, sh))
    for av in st['out_avals']:
        z = np.zeros((N_CORES * av.shape[0], *av.shape[1:]), av.dtype)
        args.append(jax.device_put(z, sh))
    return args


def _run(args):
    import jax
    st = _get_exec()
    outs = st['fn'](*args)
    jax.block_until_ready(outs)
    return outs


def kernel(feats, feats_mask, w_ih_lr, w_hh_lr, b_ih_lr, b_hh_lr,
           w_ih_rl, w_hh_rl, b_ih_rl, b_hh_rl):
    """Full-input bidirectional GRU on 8 NeuronCores (batch data-parallel).

    feats_mask is all-ones for this problem spec and is not used on device.
    """
    in_maps = prep_inputs(feats, w_ih_lr, w_hh_lr, b_ih_lr, b_hh_lr,
                          w_ih_rl, w_hh_rl, b_ih_rl, b_hh_rl, n_cores=N_CORES)
    args = _stage_inputs(in_maps)
    outs = _run(args)
    st = _STATE
    oi = st['out_names'].index('out')
    full = np.asarray(outs[oi])  # [N_CORES*B_SH, L, 2H] (batch-concat)
    return full
